# revision 18
# baseline (speedup 1.0000x reference)
"""Trainium2 Bass kernel for the sliding-window CNN problem.

Computes, for x[B=32, WORDS=512, E=256], W[1024, 1280], b[1024]:
    z[b,t,h] = sum_{w<5, e<256} x[b, t+w, e] * W[h, w*256+e]   (T = 508 windows)
    out[b,h] = relu(max_t z[b,t,h] + b[h])

Strategy: data-parallel over batch (4 batches per core, 8 cores).
Per core the window conv is 10 accumulating matmuls (5 window shifts x 2
feature chunks of 128) per [128h x 508t] PSUM tile; the window shift is a
free SBUF column offset on the moving operand.  fp16 operands (same PE
rate as bf16, ~8x better accuracy), fp32 PSUM accumulation.

Schedule notes (from perfetto trace of the previous version):
- The matmul stream runs back-to-back at 214ns/matmul (full 2.4 GHz), so
  the only recoverable time is at the head (DMA arrival + PE p-state
  ramp) and tail (post-processing after the last matmul).
- Contraction chunks are enumerated w-major within each 128-feature
  half (kc = ec*WIN + w), so the first 5 chunks all read the same
  512-column x half: only wt[0] + the first x half gate the stream
  start, and the second x half isn't needed until t0+8.6us.
- wt[0] and x[0] front-half go first on the Sync DMA queue while the
  Scalar queue carries wt[1]/wt[3] in parallel; remaining tiles stream
  in consumption order.  ~28 junk matmuls bridge the DMA wait and ramp
  the PE clock without delaying the real stream.
- Last batch runs hc-outer so each group's reduce/activation overlaps
  the next group's matmuls; the final hc group is split into two
  254-column PSUM tiles so its first half is reduced while the second
  half's matmuls still run, leaving only a 254-col reduce + two tiny
  DVE ops + one 512B DMA after the last matmul.
"""

import numpy as np

import concourse.bacc as bacc
import concourse.mybir as mybir
import concourse.tile as tile
from concourse.bass_utils import run_bass_kernel_spmd

B, WORDS, E = 32, 512, 256
WIN = 5
HIDDEN = 1024
T = WORDS - WIN + 1          # 508 sliding windows
NCORES = 8
BPC = B // NCORES            # 4 batches per core
F = WIN * E                  # 1280 contraction features
KC = F // 128                # 10 contraction chunks
HC = HIDDEN // 128           # 8 hidden chunks
EC = E // 128                # 2 feature chunks per window position
JUNK = 56                    # PE p-state warm-up matmuls
TA = T // 2                  # first-half columns of the final group

FP16 = mybir.dt.float16
FP32 = mybir.dt.float32

_CACHE = {}


def _build():
    nc = bacc.Bacc(None, target_bir_lowering=False)
    # xT[p, b, ec, t] = x[b, t, ec*128+p]
    xT = nc.dram_tensor("xT", [128, BPC, EC, WORDS], FP16, kind="ExternalInput")
    # wT[p, kc, h] = W[h, w*256 + ec*128 + p] with (ec, w) = divmod(kc, WIN)
    wT = nc.dram_tensor("wT", [128, KC, HIDDEN], FP16, kind="ExternalInput")
    bias = nc.dram_tensor("bias", [128, HC], FP32, kind="ExternalInput")
    # out[b, p, hc] = result for batch b, hidden unit hc*128+p
    out = nc.dram_tensor("out", [BPC, 128, HC], FP32, kind="ExternalOutput")

    with tile.TileContext(nc) as tc:
        with (
            tc.tile_pool(name="xin", bufs=1) as xpool,
            tc.tile_pool(name="wgt", bufs=1) as wpool,
            tc.tile_pool(name="ps", bufs=1, space="PSUM") as pspool,
            tc.tile_pool(name="post", bufs=2) as postpool,
            tc.tile_pool(name="cst", bufs=1) as cstpool,
        ):
            # DMA issue order == consumption order, with the stream-gating
            # set (wt[0] for hc=0 + the first x half) fine-sliced across
            # both HWDGE queues: the 16 DMA engines serve both queues, and
            # first-byte latency is ~1-2us after issue, so small leading
            # pieces pull the first matmul's data in ~1.5us earlier.
            wt = [
                wpool.tile([128, HIDDEN], FP16, tag=f"w_{kc}", name=f"w_{kc}")
                for kc in range(KC)
            ]
            xt = [
                xpool.tile([128, EC * WORDS], FP16, tag=f"x_{b}", name=f"x_{b}")
                for b in range(BPC)
            ]
            # Sync issues+transfers just keep ahead of the PE's consumption;
            # Scalar's transfers start ~2.1us after issue (vs ~0.9us on
            # Sync), so it only carries the odd wt chunks with >2us slack.
            nc.sync.dma_start(xt[0][:, 0:256], xT[:, 0, 0, 0:256])
            nc.scalar.dma_start(wt[0][:, 0:128], wT[:, 0, 0:128])
            nc.sync.dma_start(xt[0][:, 256:WORDS], xT[:, 0, 0, 256:WORDS])
            nc.scalar.dma_start(wt[0][:, 128:512], wT[:, 0, 128:512])
            nc.sync.dma_start(wt[0][:, 512:HIDDEN], wT[:, 0, 512:HIDDEN])
            nc.scalar.dma_start(wt[1][:], wT[:, 1])
            nc.sync.dma_start(wt[2][:], wT[:, 2])
            nc.scalar.dma_start(wt[3][:], wT[:, 3])
            nc.sync.dma_start(wt[4][:], wT[:, 4])
            nc.sync.dma_start(xt[0][:, WORDS:2 * WORDS], xT[:, 0, 1])
            for kc in range(5, KC):
                nc.sync.dma_start(wt[kc][:], wT[:, kc])
            bias_sb = cstpool.tile([128, HC], FP32, tag="bias")
            nc.sync.dma_start(bias_sb[:], bias[:])
            for b in range(1, BPC):
                nc.sync.dma_start(xt[b][:], xT[:, b])

            # PE pre-warm: junk matmuls bridge the first-DMA wait so the
            # p-state ramp (full clock after ~3us of sustained activity)
            # overlaps the transfer instead of the real stream.
            junk = cstpool.tile([128, 128], FP16, tag="junk")
            nc.gpsimd.memset(junk[:], 0.0)
            ps_junk = pspool.tile([128, 64], FP32, tag="ps7", name="ps_junk")
            for _ in range(JUNK):
                nc.tensor.matmul(
                    ps_junk[:], junk[:], junk[:, 0:64], start=True, stop=True
                )

            def emit_group(b, hc, ps):
                """All KC accumulating matmuls for psum group (b, hc)."""
                for kc in range(KC):
                    ec, w = divmod(kc, WIN)
                    base = ec * WORDS + w
                    nc.tensor.matmul(
                        ps[:],
                        wt[kc][:, hc * 128:(hc + 1) * 128],
                        xt[b][:, base: base + T],
                        start=(kc == 0),
                        stop=(kc == KC - 1),
                    )

            def emit_post(b, hc, ps, res):
                mx = postpool.tile([128, 1], FP32, tag=f"mx{hc}", name=f"mx_{b}_{hc}")
                nc.vector.reduce_max(mx[:], ps[:], axis=mybir.AxisListType.X)
                nc.scalar.activation(
                    res[:, hc:hc + 1], mx[:],
                    mybir.ActivationFunctionType.Relu,
                    bias=bias_sb[:, hc:hc + 1],
                )

            for b in range(BPC - 1):
                # kc-outer: all 8 banks accumulate in parallel; the PE's
                # weight consumption rate stays below DMA delivery, so
                # compute starts as soon as wt[0] lands.
                ps = [
                    pspool.tile([128, T], FP32, tag=f"ps{hc}", name=f"ps_{b}_{hc}")
                    for hc in range(HC)
                ]
                res = postpool.tile([128, HC], FP32, tag="res", name=f"res_{b}")
                # hc order 0,4..7,1..3: after hc=0 (its 128-col wt piece
                # arrives first), hc4-7 run off wt0h2 on the fast Sync queue
                # while the laggy Scalar piece covering hc1-3 trails in.
                for kc in range(KC):
                    ec, w = divmod(kc, WIN)
                    base = ec * WORDS + w
                    rhs = xt[b][:, base: base + T]
                    for hc in (0, 4, 5, 6, 7, 1, 2, 3):
                        nc.tensor.matmul(
                            ps[hc][:],
                            wt[kc][:, hc * 128:(hc + 1) * 128],
                            rhs,
                            start=(kc == 0),
                            stop=(kc == KC - 1),
                        )
                for hc in range(HC):
                    emit_post(b, hc, ps[hc], res)
                nc.sync.dma_start(out[b], res[:])

            # Last batch: hc-outer so groups finish staggered and the
            # reduce/act chain overlaps the remaining matmuls.  The final
            # hc group is split over T so only a half-width reduce and two
            # small DVE ops trail the last matmul; the last result ships
            # from a dedicated tile so no earlier res DMA is waited on.
            b = BPC - 1
            res = postpool.tile([128, HC], FP32, tag="res", name="res_last")
            for hc in range(HC - 1):
                psl = pspool.tile([128, T], FP32, tag=f"ps{hc}", name=f"ps_l_{hc}")
                emit_group(b, hc, psl)
                emit_post(b, hc, psl, res)
                if hc == 3:
                    nc.sync.dma_start(out[b, :, 0:4], res[:, 0:4])
            nc.sync.dma_start(out[b, :, 4:HC - 1], res[:, 4:HC - 1])

            # Final hc group, split over T into 254+127+127 so everything
            # except a 127-col reduce and two tiny DVE ops overlaps the
            # remaining matmuls.  relu(max(A,B,C)+bias) is rebuilt as
            # max(relu(max(A,B)+bias), relu(C+bias)) — relu and max commute.
            hc = HC - 1
            hsl = slice(hc * 128, (hc + 1) * 128)
            TB = (T - TA) // 2
            splits = [(0, TA, "ps7"), (TA, TA + TB, "ps0"), (TA + TB, T, "ps1")]
            mxs = []
            for si, (lo, hi, tag) in enumerate(splits):
                psl = pspool.tile([128, hi - lo], FP32, tag=tag, name=f"ps_l7{si}")
                for kc in range(KC):
                    ec, w = divmod(kc, WIN)
                    base = ec * WORDS + w + lo
                    nc.tensor.matmul(
                        psl[:], wt[kc][:, hsl], xt[b][:, base: base + (hi - lo)],
                        start=(kc == 0), stop=(kc == KC - 1),
                    )
                mx = postpool.tile([128, 1], FP32, tag=f"fmx{si}")
                nc.vector.reduce_max(mx[:], psl[:], axis=mybir.AxisListType.X)
                mxs.append(mx)
                if si == 1:
                    # off critical path: mab = relu(max(mxA, mxB) + bias)
                    mab0 = postpool.tile([128, 1], FP32, tag="mab0")
                    nc.vector.tensor_scalar(
                        mab0[:], mxs[1][:], mxs[0][:], None, mybir.AluOpType.max,
                    )
                    mab = postpool.tile([128, 1], FP32, tag="mab")
                    nc.vector.tensor_scalar(
                        mab[:], mab0[:], bias_sb[:, hc:hc + 1], 0.0,
                        mybir.AluOpType.add, mybir.AluOpType.max,
                    )
            # fres = max(mxC + bias, mab); mab >= 0 already supplies the relu
            fres = postpool.tile([128, 1], FP32, tag="fres")
            nc.vector.tensor_scalar(
                fres[:], mxs[2][:], bias_sb[:, hc:hc + 1], mab[:],
                mybir.AluOpType.add, mybir.AluOpType.max,
            )
            nc.sync.dma_start(out[b, :, hc:hc + 1], fres[:])
    nc.finalize()
    return nc


def _prep(input, W, b):
    x = np.asarray(input, dtype=np.float32)
    # x[b, t, e] -> xT[p, b, ec, t] = x[b, t, ec*128+p]
    y = x.transpose(2, 0, 1).reshape(EC, 128, B, WORDS)      # [ec, p, b, t]
    xT = np.ascontiguousarray(y.transpose(1, 2, 0, 3)).astype(np.float16)  # [p,b,ec,t]
    # W[h, f] -> wT[p, kc, h] = W[h, w*256 + ec*128 + p], kc = ec*WIN + w
    wt = np.asarray(W, dtype=np.float32).T.reshape(WIN, EC, 128, HIDDEN)
    wT = np.ascontiguousarray(wt.transpose(2, 1, 0, 3).reshape(128, KC, HIDDEN))
    wT = wT.astype(np.float16)
    # b[h] -> bias[p, hc] = b[hc*128+p]
    bias = np.ascontiguousarray(np.asarray(b, np.float32).reshape(HC, 128).T)
    return xT, wT, bias


def run(inputs, trace=False, **kwargs):
    if "nc" not in _CACHE:
        _CACHE["nc"] = _build()
    nc = _CACHE["nc"]
    xT, wT, bias = _prep(inputs["input"], inputs["W"], inputs["b"])
    in_maps = [
        {"xT": xT[:, c * BPC:(c + 1) * BPC], "wT": wT, "bias": bias}
        for c in range(NCORES)
    ]
    in_maps = [{k: np.ascontiguousarray(v) for k, v in m.items()} for m in in_maps]
    res = run_bass_kernel_spmd(nc, in_maps, list(range(NCORES)), trace=trace, **kwargs)
    # out[b, p, hc] -> full[core*BPC + b, hc*128 + p]
    parts = []
    for c in range(NCORES):
        o = res.results[c]["out"]              # [BPC, 128, HC]
        parts.append(o.transpose(0, 2, 1).reshape(BPC, HIDDEN))
    full = np.concatenate(parts, axis=0).astype(np.float32)
    return full, res


def kernel(**inputs):
    out, _ = run(inputs, trace=False)
    return out
